# revision 16
# baseline (speedup 1.0000x reference)
"""Trainium2 Bass kernel for the PerforantHebb AHA module.

Math (reference.py):
    pre     = ec @ W^T                      (B, CA3)
    targets = dg + pre                      -> output pre_pc_cue
    outer   = targets^T @ ec / B            (CA3, EC)
    m       = mean(targets, 0)              (CA3,)
    delta   = ec @ outer^T - pre * m[None]  (B, CA3)   [dW folded algebraically]
    ec_ca3_loss = pc_cue_loss = LR^2 * mean(delta^2);  dg_ca3_loss = 0

Sharding: CA3 split 8 ways (256 rows/core). Everything is local per core —
no collectives. Each core runs three K-contracted matmul phases:
  M1: preT_s  = (W_s ec^T)      : lhsT=W_sT tiles,   rhs=ecT tiles   (K=EC)
  M2: outerT_s = ec^T targets_s : lhsT=ec tiles,     rhs=targets_s   (K=B)
  M3: deltaT_s = outer_s ec^T   : lhsT=outerT tiles, rhs=ecT tiles   (K=EC)
Host feeds ec in both layouts (natural + transposed) plus per-core W_sT /
dgT slices; output slice is produced transposed and fixed up on host.
"""

import numpy as np
import ml_dtypes

import concourse.bass as bass
import concourse.bacc as bacc
import concourse.mybir as mybir
import concourse.tile as tile
from concourse.bass_utils import run_bass_kernel_spmd
from concourse.masks import make_identity

B, EC, CA3 = 1024, 4096, 2048
LR = 0.01
N_CORES = 8
S = CA3 // N_CORES          # 256 ca3 rows per core
P = 128
KT = EC // P                # 32 k-tiles over EC
NB = B // P                 # 8 b-chunks of 128
BB = B // 512               # 2 b-chunks of 512
C2 = S // P                 # 2 ca3 partition tiles

F32 = mybir.dt.float32
MMDT = mybir.dt.bfloat16    # matmul operand dtype
MMNP = ml_dtypes.bfloat16

_CACHE = {}


def _build(phases=4):
    import os
    phases = int(os.environ.get("KERNEL_PHASES", phases))
    nc = bacc.Bacc("TRN2", target_bir_lowering=False, debug=False,
                   num_devices=N_CORES)
    ecT = nc.dram_tensor("ecT", (EC, B), MMDT, kind="ExternalInput").ap()
    ecN = nc.dram_tensor("ecN", (B, EC), MMDT, kind="ExternalInput").ap()
    WT = nc.dram_tensor("WT", (EC, S), MMDT, kind="ExternalInput").ap()
    dgT = nc.dram_tensor("dgT", (S, B), F32, kind="ExternalInput").ap()
    outT = nc.dram_tensor("outT", (S, B), F32, kind="ExternalOutput").ap()
    loss = nc.dram_tensor("loss", (P, 1), F32, kind="ExternalOutput").ap()

    with tile.TileContext(nc) as tc:
        with (
            tc.tile_pool(name="res", bufs=1) as res,
            tc.tile_pool(name="tmp", bufs=4) as tmp,
            tc.tile_pool(name="ps", bufs=4, space="PSUM") as ps,
            tc.tile_pool(name="pst", bufs=2, space="PSUM") as pst,
        ):
            # ---- resident loads ----
            sb_ecT = [res.tile([P, B], MMDT, tag=f"ecT{k}", name=f"ecT{k}") for k in range(KT)]
            sb_WT = [res.tile([P, S], MMDT, tag=f"WT{k}", name=f"WT{k}") for k in range(KT)]
            sb_ecN = [res.tile([P, EC], MMDT, tag=f"ecN{b}", name=f"ecN{b}") for b in range(NB)]
            sb_dgT = [res.tile([P, B], F32, tag=f"dgT{c}", name=f"dgT{c}") for c in range(C2)]
            # dgT early on the gpsimd (SWDGE) queue so M1 epilogues never wait;
            # ecN also via gpsimd so it streams in parallel with the sync-queue
            # ecT/WT feed that paces M1.
            for c in range(C2):
                nc.gpsimd.dma_start(sb_dgT[c][:], dgT[c * P:(c + 1) * P, :])
            for k in range(KT):
                nc.sync.dma_start(sb_ecT[k][:], ecT[k * P:(k + 1) * P, :])
                nc.sync.dma_start(sb_WT[k][:], WT[k * P:(k + 1) * P, :])
            for b in range(NB):
                nc.gpsimd.dma_start(sb_ecN[b][:], ecN[b * P:(b + 1) * P, :])

            identity = res.tile([P, P], F32, tag="ident", name="ident")
            make_identity(nc, identity[:])

            sb_tT = [res.tile([P, B], F32, tag=f"tT{c}", name=f"tT{c}") for c in range(C2)]
            sb_tb = [res.tile([P, S], MMDT, tag=f"tb{b}", name=f"tb{b}") for b in range(NB)]
            sb_oT = [res.tile([P, S], MMDT, tag=f"oT{k}", name=f"oT{k}") for k in range(KT)]
            sb_pm = [res.tile([P, B], F32, tag=f"pm{c}", name=f"pm{c}") for c in range(C2)]
            sb_m = res.tile([P, C2], F32, tag="m", name="m")

            # ---- M1: preT(c2) tiles = sum_k W_sT[k,c2]^T @ ecT[k,bb] ----
            # k-outer: all 4 psum groups advance per k-tile, so the PE issues
            # 4 matmuls per arriving k-tile DMA and finishes with the stream.
            p1 = [[ps.tile([P, 512], F32, tag="mm", name="mmps")
                   for _ in range(BB)] for _ in range(C2)]
            for k in range(KT):
                for c in range(C2):
                    for bb in range(BB):
                        nc.tensor.matmul(
                            p1[c][bb][:], sb_WT[k][:, c * P:(c + 1) * P],
                            sb_ecT[k][:, bb * 512:(bb + 1) * 512],
                            start=(k == 0), stop=(k == KT - 1))
            for c in range(C2):
                for bb in range(BB):
                    sl = slice(bb * 512, (bb + 1) * 512)
                    # targetsT = preT + dgT  (f32, also the main output)
                    nc.vector.tensor_add(
                        sb_tT[c][:, sl], p1[c][bb][:], sb_dgT[c][:, sl])
                    nc.sync.dma_start(outT[c * P:(c + 1) * P, sl], sb_tT[c][:, sl])

            # ---- transpose targetsT -> targets natural (bf16) for M2 rhs ----
            for b in range(NB if phases >= 2 else 0):
                for c in range(C2):
                    pt = pst.tile([P, P], F32, tag="tr", name="trps")
                    nc.tensor.transpose(
                        pt[:], sb_tT[c][:, b * P:(b + 1) * P], identity[:])
                    nc.vector.tensor_copy(sb_tb[b][:, c * P:(c + 1) * P], pt[:])

            # ---- m(c2) = rowsum_B(targetsT); pm = (targetsT - dgT) * m / B ----
            # (after the transposes in DVE program order: overlaps M2)
            for c in range(C2 if phases >= 2 else 0):
                nc.vector.tensor_reduce(
                    sb_m[:, c:c + 1], sb_tT[c][:], axis=mybir.AxisListType.X,
                    op=mybir.AluOpType.add)
            for c in range(C2 if phases >= 2 else 0):
                nc.vector.tensor_sub(sb_pm[c][:], sb_tT[c][:], sb_dgT[c][:])
                nc.vector.tensor_scalar(
                    sb_pm[c][:], sb_pm[c][:], sb_m[:, c:c + 1], 1.0 / B,
                    op0=mybir.AluOpType.mult, op1=mybir.AluOpType.mult)

            # ---- M2: outerT[e] = sum_b ecN[b,e]^T @ targets[b], scaled 1/B ----
            for e in range(KT if phases >= 3 else 0):
                p2 = ps.tile([P, 512], F32, tag="mm", name="mmps")
                for b in range(NB):
                    nc.tensor.matmul(
                        p2[:, :S], sb_ecN[b][:, e * P:(e + 1) * P], sb_tb[b][:],
                        start=(b == 0), stop=(b == NB - 1))
                nc.vector.tensor_scalar_mul(sb_oT[e][:], p2[:, :S], 1.0 / B)

            # ---- M3: deltaT(c2,bb) = sum_k outerT[k,c2]^T @ ecT[k,bb] - pm ----
            part_prev = None
            for c in range(C2 if phases >= 4 else 0):
                for bb in range(BB):
                    p3 = ps.tile([P, 512], F32, tag="mm", name="mmps")
                    for k in range(KT):
                        nc.tensor.matmul(
                            p3[:], sb_oT[k][:, c * P:(c + 1) * P],
                            sb_ecT[k][:, bb * 512:(bb + 1) * 512],
                            start=(k == 0), stop=(k == KT - 1))
                    sl = slice(bb * 512, (bb + 1) * 512)
                    d = tmp.tile([P, 512], F32, tag="d", name="d")
                    nc.vector.tensor_sub(d[:], p3[:], sb_pm[c][:, sl])
                    part = tmp.tile([P, 1], F32, tag=f"part{c}_{bb}", name=f"part{c}_{bb}")
                    if int(os.environ.get("KERNEL_TTR", "0")):
                        dsq = tmp.tile([P, 512], F32, tag="dsq", name="dsq")
                        nc.vector.tensor_tensor_reduce(
                            dsq[:], d[:], d[:], 1.0,
                            0.0 if part_prev is None else part_prev[:],
                            op0=mybir.AluOpType.mult, op1=mybir.AluOpType.add,
                            accum_out=part[:])
                    else:
                        dsq = tmp.tile([P, 512], F32, tag="dsq", name="dsq")
                        nc.vector.tensor_mul(dsq[:], d[:], d[:])
                        nc.vector.tensor_reduce(
                            part[:], dsq[:], axis=mybir.AxisListType.X,
                            op=mybir.AluOpType.add)
                        if part_prev is not None:
                            nc.vector.tensor_add(part[:], part[:], part_prev[:])
                    part_prev = part

            # ---- write per-partition loss partials; host sums them ----
            if part_prev is not None:
                nc.sync.dma_start(loss[:, :], part_prev[:])
            else:
                zl = tmp.tile([P, 1], F32, tag="zl", name="zl")
                nc.vector.memset(zl[:], 0.0)
                nc.sync.dma_start(loss[:, :], zl[:])

    nc.compile()
    return nc


def _get_nc():
    if "nc" not in _CACHE:
        _CACHE["nc"] = _build()
    return _CACHE["nc"]


def kernel(ec_inputs: np.ndarray, dg_inputs: np.ndarray, W_ec: np.ndarray):
    nc = _get_nc()
    ec = np.asarray(ec_inputs, dtype=np.float32)
    dg = np.asarray(dg_inputs, dtype=np.float32)
    W = np.asarray(W_ec, dtype=np.float32)

    ecN_h = ec.astype(MMNP)
    ecT_h = np.ascontiguousarray(ec.T).astype(MMNP)
    in_maps = []
    for i in range(N_CORES):
        Ws = W[i * S:(i + 1) * S, :]
        in_maps.append({
            "ecT": ecT_h,
            "ecN": ecN_h,
            "WT": np.ascontiguousarray(Ws.T).astype(MMNP),
            "dgT": np.ascontiguousarray(dg[:, i * S:(i + 1) * S].T),
        })

    res = run_bass_kernel_spmd(nc, in_maps, list(range(N_CORES)))

    pre_pc_cue = np.empty((B, CA3), dtype=np.float32)
    loss_sum = 0.0
    for i in range(N_CORES):
        pre_pc_cue[:, i * S:(i + 1) * S] = res.results[i]["outT"].T
        loss_sum += float(res.results[i]["loss"].sum())

    loss = np.float32(LR * LR * loss_sum / (B * CA3))
    return (pre_pc_cue, np.float32(0.0), loss, loss)


# revision 20
# speedup vs baseline: 1.2028x; 1.2028x over previous
"""Trainium2 Bass kernel for the PerforantHebb AHA module.

Math (reference.py):
    pre     = ec @ W^T                      (B, CA3)
    targets = dg + pre                      -> output pre_pc_cue
    outer   = targets^T @ ec / B            (CA3, EC)
    m       = mean(targets, 0)              (CA3,)
    delta   = ec @ outer^T - pre * m[None]  (B, CA3)   [dW folded algebraically]
    ec_ca3_loss = pc_cue_loss = LR^2 * mean(delta^2);  dg_ca3_loss = 0

Sharding: CA3 split 8 ways (256 rows/core). Everything is local per core —
no collectives. Each core runs three K-contracted matmul phases:
  M1: preT_s  = (W_s ec^T)      : lhsT=W_sT tiles,   rhs=ecT tiles   (K=EC)
  M2: outerT_s = ec^T targets_s : lhsT=ec tiles,     rhs=targets_s   (K=B)
  M3: deltaT_s = outer_s ec^T   : lhsT=outerT tiles, rhs=ecT tiles   (K=EC)
Host feeds ec in both layouts (natural + transposed) plus per-core W_sT /
dgT slices; output slice is produced transposed and fixed up on host.
"""

import numpy as np
import ml_dtypes

import concourse.bass as bass
import concourse.bacc as bacc
import concourse.mybir as mybir
import concourse.tile as tile
from concourse.bass_utils import run_bass_kernel_spmd
from concourse.masks import make_identity

B, EC, CA3 = 1024, 4096, 2048
LR = 0.01
N_CORES = 8
S = CA3 // N_CORES          # 256 ca3 rows per core
P = 128
KT = EC // P                # 32 k-tiles over EC
NB = B // P                 # 8 b-chunks of 128
BB = B // 512               # 2 b-chunks of 512
C2 = S // P                 # 2 ca3 partition tiles

F32 = mybir.dt.float32
MMDT = mybir.dt.bfloat16    # matmul operand dtype
MMNP = ml_dtypes.bfloat16

_CACHE = {}


def _build(phases=4):
    import os
    phases = int(os.environ.get("KERNEL_PHASES", phases))
    nc = bacc.Bacc("TRN2", target_bir_lowering=False, debug=False,
                   num_devices=N_CORES)
    ecT = nc.dram_tensor("ecT", (EC, B), MMDT, kind="ExternalInput").ap()
    ecN = nc.dram_tensor("ecN", (B, EC), MMDT, kind="ExternalInput").ap()
    WT = nc.dram_tensor("WT", (EC, S), MMDT, kind="ExternalInput").ap()
    dgT = nc.dram_tensor("dgT", (S, B), F32, kind="ExternalInput").ap()
    outT = nc.dram_tensor("outT", (S, B), F32, kind="ExternalOutput").ap()
    loss = nc.dram_tensor("loss", (P, 1), F32, kind="ExternalOutput").ap()

    KP = KT // 2  # k-pair granules for batched DMA

    with tile.TileContext(nc) as tc:
        with (
            tc.tile_pool(name="res", bufs=1) as res,
            tc.tile_pool(name="tmp", bufs=4) as tmp,
            tc.tile_pool(name="ps", bufs=8, space="PSUM") as ps,
        ):
            # ---- resident tiles ----
            sb_ecT = [res.tile([P, 2, B], MMDT, tag=f"ecT{kp}", name=f"ecT{kp}")
                      for kp in range(KP)]
            sb_WT = [res.tile([P, 2, S], MMDT, tag=f"WT{kp}", name=f"WT{kp}")
                     for kp in range(KP)]
            sb_ecN = [res.tile([P, EC], MMDT, tag=f"ecN{b}", name=f"ecN{b}")
                      for b in range(NB)]
            sb_dgT = [res.tile([P, B], F32, tag=f"dgT{c}", name=f"dgT{c}")
                      for c in range(C2)]

            # dgT early on the gpsimd (SWDGE) queue so M1 epilogues never wait.
            for c in range(C2):
                nc.gpsimd.dma_start(sb_dgT[c][:], dgT[c * P:(c + 1) * P, :])
            # Input stream striped across TWO HWDGE queues (sync + scalar) to
            # engage more SDMA engines: ecT/WT pairs first (they pace M1),
            # then ecN chunks (they pace M2's first quarter). Output writes go
            # on the gpsimd queue to stay clear of the input streams.
            qs = [nc.sync, nc.scalar]
            for kp in range(KP):
                q = qs[kp % 2]
                q.dma_start(
                    sb_ecT[kp][:],
                    ecT[kp * 2 * P:(kp + 1) * 2 * P, :].rearrange(
                        "(two p) b -> p two b", p=P))
                q.dma_start(
                    sb_WT[kp][:],
                    WT[kp * 2 * P:(kp + 1) * 2 * P, :].rearrange(
                        "(two p) s -> p two s", p=P))
            for b in range(NB):
                qs[b % 2].dma_start(sb_ecN[b][:], ecN[b * P:(b + 1) * P, :])

            identity = res.tile([P, P], F32, tag="ident", name="ident")
            make_identity(nc, identity[:])

            sb_tT = [res.tile([P, B], F32, tag=f"tT{c}", name=f"tT{c}")
                     for c in range(C2)]
            sb_tb = [res.tile([P, S], MMDT, tag=f"tb{b}", name=f"tb{b}")
                     for b in range(NB)]
            sb_oT = [res.tile([P, S], MMDT, tag=f"oT{k}", name=f"oT{k}")
                     for k in range(KT)]
            sb_pm = [res.tile([P, B], F32, tag=f"pm{c}", name=f"pm{c}")
                     for c in range(C2)]
            sb_m = res.tile([P, C2], F32, tag="m", name="m")

            def ecT_sl(k, bb):
                return sb_ecT[k // 2][:, k % 2, bb * 512:(bb + 1) * 512]

            def WT_sl(k, c):
                return sb_WT[k // 2][:, k % 2, c * P:(c + 1) * P]

            # ---- M1: k-outer, 4 psum groups advance per arriving k-tile ----
            p1 = [[ps.tile([P, 512], F32, tag="mm", name="mmps")
                   for _ in range(BB)] for _ in range(C2)]
            for k in range(KT):
                for c in range(C2):
                    for bb in range(BB):
                        nc.tensor.matmul(
                            p1[c][bb][:], WT_sl(k, c), ecT_sl(k, bb),
                            start=(k == 0), stop=(k == KT - 1))
            for c in range(C2):
                for bb in range(BB):
                    sl = slice(bb * 512, (bb + 1) * 512)
                    # targetsT = preT + dgT  (f32, also the main output)
                    nc.vector.tensor_add(
                        sb_tT[c][:, sl], p1[c][bb][:], sb_dgT[c][:, sl])
                    nc.gpsimd.dma_start(outT[c * P:(c + 1) * P, sl],
                                        sb_tT[c][:, sl])

            # ---- transpose targetsT -> targets natural (bf16) for M2 rhs ----
            for b in range(NB if phases >= 2 else 0):
                for c in range(C2):
                    pt = ps.tile([P, 512], F32, tag="mm", name="trps")
                    nc.tensor.transpose(
                        pt[:, :P], sb_tT[c][:, b * P:(b + 1) * P], identity[:])
                    nc.vector.tensor_copy(sb_tb[b][:, c * P:(c + 1) * P],
                                          pt[:, :P])

            # ---- m(c2) = rowsum_B(targetsT); pm = (targetsT - dgT) * m / B ----
            # (after the transposes in DVE program order: overlaps M2)
            for c in range(C2 if phases >= 2 else 0):
                nc.vector.tensor_reduce(
                    sb_m[:, c:c + 1], sb_tT[c][:], axis=mybir.AxisListType.X,
                    op=mybir.AluOpType.add)
            for c in range(C2 if phases >= 2 else 0):
                nc.vector.tensor_sub(sb_pm[c][:], sb_tT[c][:], sb_dgT[c][:])
                nc.vector.tensor_scalar(
                    sb_pm[c][:], sb_pm[c][:], sb_m[:, c:c + 1], 1.0 / B,
                    op0=mybir.AluOpType.mult, op1=mybir.AluOpType.mult)

            # ---- M2: outerT[e] = sum_b ecN[b,e]^T @ targets[b], scaled 1/B.
            # Four e-quarters of 8; within a quarter, b-outer with 8
            # concurrent psum groups (one per bank — PSUM allows only one
            # accumulation group per 2KiB zero region), so the first
            # quarter's MMs chase the ecN DMA stream. ----
            EH = 8
            for h in range(4 if phases >= 3 else 0):
                p2 = [ps.tile([P, 512], F32, tag="mm", name="mmps")
                      for _ in range(EH)]
                for b in range(NB):
                    for ei in range(EH):
                        e = h * EH + ei
                        nc.tensor.matmul(
                            p2[ei][:, :S], sb_ecN[b][:, e * P:(e + 1) * P],
                            sb_tb[b][:],
                            start=(b == 0), stop=(b == NB - 1))
                for ei in range(EH):
                    e = h * EH + ei
                    nc.vector.tensor_scalar_mul(
                        sb_oT[e][:], p2[ei][:, :S], 1.0 / B)

            # ---- M3: deltaT(c2,bb) = sum_k outerT[k,c2]^T @ ecT[k,bb] - pm ----
            part_prev = None
            for c in range(C2 if phases >= 4 else 0):
                for bb in range(BB):
                    p3 = ps.tile([P, 512], F32, tag="mm", name="mmps")
                    for k in range(KT):
                        nc.tensor.matmul(
                            p3[:], sb_oT[k][:, c * P:(c + 1) * P], ecT_sl(k, bb),
                            start=(k == 0), stop=(k == KT - 1))
                    sl = slice(bb * 512, (bb + 1) * 512)
                    d = tmp.tile([P, 512], F32, tag="d", name="d")
                    nc.vector.tensor_sub(d[:], p3[:], sb_pm[c][:, sl])
                    part = tmp.tile([P, 1], F32, tag=f"part{c}_{bb}",
                                    name=f"part{c}_{bb}")
                    dsq = tmp.tile([P, 512], F32, tag="dsq", name="dsq")
                    nc.vector.tensor_mul(dsq[:], d[:], d[:])
                    nc.vector.tensor_reduce(
                        part[:], dsq[:], axis=mybir.AxisListType.X,
                        op=mybir.AluOpType.add)
                    if part_prev is not None:
                        nc.vector.tensor_add(part[:], part[:], part_prev[:])
                    part_prev = part

            # ---- write per-partition loss partials; host sums them ----
            if part_prev is not None:
                nc.scalar.dma_start(loss[:, :], part_prev[:])
            else:
                zl = tmp.tile([P, 1], F32, tag="zl", name="zl")
                nc.vector.memset(zl[:], 0.0)
                nc.scalar.dma_start(loss[:, :], zl[:])

    nc.compile()
    return nc


def _get_nc():
    if "nc" not in _CACHE:
        _CACHE["nc"] = _build()
    return _CACHE["nc"]


def kernel(ec_inputs: np.ndarray, dg_inputs: np.ndarray, W_ec: np.ndarray):
    nc = _get_nc()
    ec = np.asarray(ec_inputs, dtype=np.float32)
    dg = np.asarray(dg_inputs, dtype=np.float32)
    W = np.asarray(W_ec, dtype=np.float32)

    ecN_h = ec.astype(MMNP)
    ecT_h = np.ascontiguousarray(ec.T).astype(MMNP)
    in_maps = []
    for i in range(N_CORES):
        Ws = W[i * S:(i + 1) * S, :]
        in_maps.append({
            "ecT": ecT_h,
            "ecN": ecN_h,
            "WT": np.ascontiguousarray(Ws.T).astype(MMNP),
            "dgT": np.ascontiguousarray(dg[:, i * S:(i + 1) * S].T),
        })

    res = run_bass_kernel_spmd(nc, in_maps, list(range(N_CORES)))

    pre_pc_cue = np.empty((B, CA3), dtype=np.float32)
    loss_sum = 0.0
    for i in range(N_CORES):
        pre_pc_cue[:, i * S:(i + 1) * S] = res.results[i]["outT"].T
        loss_sum += float(res.results[i]["loss"].sum())

    loss = np.float32(LR * LR * loss_sum / (B * CA3))
    return (pre_pc_cue, np.float32(0.0), loss, loss)
